# revision 20
# baseline (speedup 1.0000x reference)
"""DiT attention (B=2, T=2048, D=1024, H=16, rope on head 0) on 8 trn2 cores.

Sharding: tensor-parallel over heads. Core c owns heads {2c, 2c+1}:
  - QKV projection: column-sharded (384 features per core), x^T replicated
    (pre-tiled on host, bf16, one resident SBUF tile per 512-token group).
  - Q^T/K^T kept transposed [dims, tokens]; V projected directly in natural
    [tokens, dims] layout (x^T tile as the stationary matmul operand), with a
    resident ones-column per key tile for the softmax denominator.
  - Attention fully local per (batch, head); scores computed per 128-key tile
    (S^T = K^T' @ Q^T), exp evicts PSUM->SBUF bf16 on the ACT engine; row
    sums ride along as psum row 64 of the AV accumulation.
  - Softmax denominator: DVE reciprocal + gpsimd partition_broadcast (no DMA).
  - Out projection row-sharded; per-core partial written bf16, summed on host.
Everything on the PE runs bf16 (1 cycle/row); the whole schedule is one
software-pipelined stream: each batch's QKV projection is interleaved into
the previous attention work so the PE fills exp-latency gaps and the ACT
engine (exp, the second-busiest floor) never starves.
"""
import sys
sys.path.insert(0, "/opt/trn_rl_repo")
import numpy as np

B, T, D, H, HD = 2, 2048, 1024, 16, 64
NCORES = 8
NTOK = B * T            # 4096
NG = 8                  # 512-token groups (b*4 + tt)
KC = 8                  # contraction chunks of 128 over D
NKT = T // 128          # 16 key tiles per batch
QC = 4                  # 512-query chunks per batch
ROPE_BASE = 10000.0

_CACHE = {}


def _build():
    import concourse.bacc as bacc
    import concourse.mybir as mybir
    import concourse.tile as tile

    F32 = mybir.dt.float32
    BF16 = mybir.dt.bfloat16
    EXP = mybir.ActivationFunctionType.Exp

    nc = bacc.Bacc("TRN2", target_bir_lowering=False, debug=False, num_devices=NCORES)

    xt_d = nc.dram_tensor("xt", [128, NG * KC * 512], BF16, kind="ExternalInput")
    wqkv = nc.dram_tensor("wqkv", [128, KC * 384], BF16, kind="ExternalInput")
    wout = nc.dram_tensor("wout", [128, D], BF16, kind="ExternalInput")
    cosT = nc.dram_tensor("cosT", [64, T], BF16, kind="ExternalInput")
    sinT = nc.dram_tensor("sinT", [64, T], BF16, kind="ExternalInput")
    maskb = nc.dram_tensor("maskb", [128, B * NKT], F32, kind="ExternalInput")
    out_d = nc.dram_tensor("out", [128, B * QC * 4096], BF16, kind="ExternalOutput")

    with tile.TileContext(nc) as tc:
        with (
            tc.tile_pool(name="consts", bufs=1) as consts,
            tc.tile_pool(name="resid", bufs=1) as resid,
            tc.tile_pool(name="xtp", bufs=1) as xtp,
            tc.tile_pool(name="ptp", bufs=3) as ptp,
            tc.tile_pool(name="rotp", bufs=2) as rotp,
            tc.tile_pool(name="smallp", bufs=2) as smallp,
            tc.tile_pool(name="outst", bufs=2) as outstp,
            tc.tile_pool(name="stgp", bufs=2) as stgp,
            tc.tile_pool(name="ps_sc", bufs=2, space="PSUM") as ps_sc,
            tc.tile_pool(name="ps_av", bufs=1, space="PSUM") as ps_av,
            tc.tile_pool(name="ps_pj", bufs=2, space="PSUM") as ps_pj,
        ):
            # ---- constants + x^T tiles, ordered to shorten the critical
            # startup chain (first scores need wq + xt0 + cos/sin) ----
            wq_sb = consts.tile([128, KC * 384], BF16)
            nc.sync.dma_start(wq_sb[:, 0:4 * 384], wqkv[:, 0:4 * 384])
            xts = [xtp.tile([128, KC * 512], BF16, name=f"xt{g}") for g in range(NG)]

            def load_xt(g):
                nc.sync.dma_start(xts[g][:, 0:2048], xt_d[:, g * 4096:g * 4096 + 2048])
                nc.sync.dma_start(xts[g][:, 2048:4096],
                                  xt_d[:, g * 4096 + 2048:(g + 1) * 4096])

            nc.sync.dma_start(xts[0][:, 0:2048], xt_d[:, 0:2048])
            nc.sync.dma_start(wq_sb[:, 4 * 384:], wqkv[:, 4 * 384:])
            nc.sync.dma_start(xts[0][:, 2048:4096], xt_d[:, 2048:4096])
            # only the first 512 cols of cos/sin block the first rope; the
            # rest can land after the next x tile
            cos_sb = consts.tile([64, T], BF16)
            nc.sync.dma_start(cos_sb[:, 0:512], cosT[:, 0:512])
            sin_sb = consts.tile([64, T], BF16)
            nc.sync.dma_start(sin_sb[:, 0:512], sinT[:, 0:512])
            mb_sb = consts.tile([128, B * NKT], F32)
            nc.sync.dma_start(mb_sb[:], maskb[:])
            load_xt(1)
            nc.sync.dma_start(cos_sb[:, 512:], cosT[:, 512:])
            nc.sync.dma_start(sin_sb[:, 512:], sinT[:, 512:])
            for g in range(2, NG):
                load_xt(g)
            wout_sb = consts.tile([128, D], BF16)
            nc.sync.dma_start(wout_sb[:], wout[:])

            # ---- resident per-batch tensors ----
            qt_sb = [resid.tile([128, T], BF16, name=f"qt{b}") for b in range(B)]
            kt_sb = [resid.tile([128, T], BF16, name=f"kt{b}") for b in range(B)]
            # V natural layout: per batch [128 keys, (h,kt) blocks of 65]
            # (col 64 of each block stays 1.0 from the initial memset -> row
            # 64 of the AV psum accumulates the softmax denominator)
            vn_sb = [resid.tile([128, 2 * NKT * 65], BF16, name=f"vn{b}") for b in range(B)]
            for b in range(B):
                nc.gpsimd.memset(vn_sb[b][:], 1.0)


            def pull(bgs, want_pe=False):
                """Advance background generators by one item; with want_pe,
                keep going until an item that issued PE work (so exp-latency
                gaps in the foreground stream get matmul filler)."""
                steps = 0
                while bgs and steps < 8:
                    try:
                        tag = next(bgs[0])
                    except StopIteration:
                        bgs.pop(0)
                        continue
                    steps += 1
                    if not want_pe or tag == "pe":
                        return

            def proj_gen(b, g):
                """QKV projection for (batch b, 512-token group g). K and Q land
                transposed [dims, tokens] (+rope on rows 0:64); V lands natural
                [tokens, dims] by using x^T as the stationary operand."""
                xt = xts[b * 4 + g]
                sl = slice(g * 512, (g + 1) * 512)
                for ft, dst in ((1, kt_sb[b]), (0, qt_sb[b])):
                    ps = ps_pj.tile([128, 512], F32, name=f"pj{b}{g}{ft}", tag="pj")
                    for kc in range(KC):
                        nc.tensor.matmul(
                            ps[:], wq_sb[:, kc * 384 + ft * 128:kc * 384 + (ft + 1) * 128],
                            xt[:, kc * 512:(kc + 1) * 512],
                            start=(kc == 0), stop=(kc == KC - 1),
                        )
                    nc.vector.tensor_copy(dst[:, sl], ps[:])
                    yield "pe"
                    # RoPE on head-even rows (identity data on cores != 0)
                    rot = rotp.tile([64, 512], BF16, name=f"rot{b}{g}{ft}", tag="rot")
                    nc.gpsimd.tensor_copy(rot[0:32, :], dst[32:64, sl])
                    nc.gpsimd.tensor_copy(rot[32:64, :], dst[0:32, sl])
                    yield
                    nc.vector.tensor_mul(rot[:], rot[:], sin_sb[:, sl])
                    nc.vector.tensor_mul(dst[0:64, sl], dst[0:64, sl], cos_sb[:, sl])
                    yield
                    nc.vector.tensor_add(dst[0:64, sl], dst[0:64, sl], rot[:])
                    yield
                psv = ps_pj.tile([128, 512], F32, name=f"pv{b}{g}", tag="pj")
                for j in range(4):
                    for kc in range(KC):
                        nc.tensor.matmul(
                            psv[:, j * 128:(j + 1) * 128],
                            xt[:, kc * 512 + j * 128:kc * 512 + (j + 1) * 128],
                            wq_sb[:, kc * 384 + 256:kc * 384 + 384],
                            start=(kc == 0), stop=(kc == KC - 1),
                        )
                    yield "pe"
                psv_r = psv[:].rearrange("p (j c) -> p j c", j=4)
                for h in range(2):
                    base = (h * NKT + g * 4) * 65
                    dst = vn_sb[b][:, base:base + 4 * 65].rearrange(
                        "p (j c) -> p j c", j=4)[:, :, 0:64]
                    nc.vector.tensor_copy(dst, psv_r[:, :, h * 64:(h + 1) * 64])
                yield

            def attn_unit(b, qc, bgs, gates):
                """Attention for (batch b, 512-query chunk qc). PE stream is
                software-pipelined (scores one key-tile ahead of AV); `gates`
                (qc==0 only) are this batch's projection generators, issued
                just-in-time before the first scores that need them; `bgs` are
                background generators (next batch's projection, previous
                chunks' normalize+out-proj tails) interleaved per key tile."""
                q0 = qc * 512
                av = [ps_av.tile([65, 512], F32, name=f"av{b}{qc}{h}", tag=f"av{h}")
                      for h in range(2)]
                sc_t = {}

                def trace_scores(kt):
                    sc = ps_sc.tile([128, 1024], F32, name=f"sc{b}{qc}{kt}", tag="sc")
                    for h in range(2):
                        nc.tensor.matmul(
                            sc[:, h * 512:(h + 1) * 512],
                            kt_sb[b][h * 64:(h + 1) * 64, kt * 128:(kt + 1) * 128],
                            qt_sb[b][h * 64:(h + 1) * 64, q0:q0 + 512],
                            start=True, stop=True,
                        )
                    sc_t[kt] = sc

                def ensure_gate(g):
                    if gates and g < len(gates) and gates[g] is not None:
                        for _ in gates[g]:
                            pull(bgs)
                        gates[g] = None

                ensure_gate(0)
                trace_scores(0)
                for kt in range(NKT):
                    if kt + 1 < NKT:
                        ensure_gate((kt + 1) // 4)
                        trace_scores(kt + 1)
                    if kt < 2:
                        # PE is in-order: queue filler BEFORE the first AV
                        # matmuls, which stall on the previous chunk's av-bank
                        # eviction
                        pull(bgs, want_pe=True)
                    pt = ptp.tile([128, 1024], BF16, name=f"pt{b}{qc}{kt}", tag="pt")
                    nc.scalar.activation(pt[:], sc_t.pop(kt)[:], EXP,
                                         bias=mb_sb[:, b * NKT + kt:b * NKT + kt + 1],
                                         scale=float(HD) ** -0.5)
                    for h in range(2):
                        nc.tensor.matmul(
                            av[h][:],
                            vn_sb[b][:, (h * NKT + kt) * 65:(h * NKT + kt + 1) * 65],
                            pt[:, h * 512:(h + 1) * 512],
                            start=(kt == 0), stop=(kt == NKT - 1),
                        )
                    pull(bgs, want_pe=True)

                # Evict AV psum right away (frees the banks for the next
                # chunk) and kick off the denominator reciprocal+broadcast;
                # normalization and out-proj are deferred to tail(). The very
                # last chunk has no successor: skip the eviction (normalize
                # straight from psum) and fan the out-proj evictions across
                # DVE+ACT so the drain is as short as possible.
                last = (b == B - 1) and (qc == QC - 1)
                av_sb, bcasts = [], []
                for h in range(2):
                    srecip = smallp.tile([1, 512], F32, name=f"sr{b}{qc}{h}", tag=f"sr{h}")
                    nc.vector.reciprocal(srecip[:], av[h][64:65, :])
                    if last:
                        av_sb.append(av[h][0:64, :])
                    else:
                        avc = smallp.tile([64, 512], F32, name=f"avc{b}{qc}{h}",
                                          tag=f"avc{h}")
                        # one head per engine so the two av banks free in
                        # parallel
                        eng = nc.scalar.copy if h == (b == 0) else \
                            nc.vector.tensor_copy
                        eng(avc[:], av[h][0:64, :])
                        av_sb.append(avc[:])
                    bc = smallp.tile([64, 512], F32, name=f"bc{b}{qc}{h}", tag=f"bc{h}")
                    nc.gpsimd.partition_broadcast(bc[:], srecip[:])
                    bcasts.append(bc)

                def tail():
                    out_st = outstp.tile([128, 512], BF16, name=f"os{b}{qc}", tag="os")
                    for h in range(2):
                        nc.vector.tensor_mul(out_st[h * 64:(h + 1) * 64, :],
                                             av_sb[h], bcasts[h][:])
                        yield
                    g2 = b * QC + qc
                    nhalf = 4 if last else 2
                    for half in range(nhalf):
                        w = 4096 // nhalf
                        stg = stgp.tile([128, w], BF16, name=f"stg{b}{qc}{half}",
                                        tag=f"stg{half % 2}")
                        for i in range(w // 512):
                            qt, nt = divmod(half * (w // 512) + i, 2)
                            po = ps_pj.tile([128, 512], F32,
                                            name=f"po{b}{qc}{qt}{nt}", tag="pj")
                            nc.tensor.matmul(
                                po[:], out_st[:, qt * 128:(qt + 1) * 128],
                                wout_sb[:, nt * 512:(nt + 1) * 512],
                                start=True, stop=True,
                            )
                            if last:
                                # drain region: halve eviction latency by
                                # fanning each across DVE + ACT
                                nc.vector.tensor_copy(
                                    stg[:, i * 512:i * 512 + 256], po[:, 0:256])
                                nc.scalar.copy(
                                    stg[:, i * 512 + 256:(i + 1) * 512], po[:, 256:512])
                            else:
                                nc.vector.tensor_copy(
                                    stg[:, i * 512:(i + 1) * 512], po[:])
                            yield "pe"
                        nc.sync.dma_start(
                            out_d[:, g2 * 4096 + half * w:g2 * 4096 + (half + 1) * w],
                            stg[:])
                        yield

                return tail()

            # ---- schedule: one interleaved stream ----
            projs = [[proj_gen(b, g) for g in range(4)] for b in range(B)]
            bgs = []
            for b in range(B):
                for qc in range(QC):
                    t = attn_unit(b, qc, bgs, projs[b] if qc == 0 else None)
                    bgs.append(t)
                if b + 1 < B:
                    # next batch's projection fills PE gaps during this
                    # batch's remaining (ACT-bound) attention chunks
                    bgs[0:0] = projs[b + 1]
            for g in bgs:
                for _ in g:
                    pass

    nc.compile()
    return nc


def _host_inputs(x, w_qkv, w_out, mask):
    import ml_dtypes
    bf = ml_dtypes.bfloat16
    x = np.asarray(x, dtype=np.float32)
    w_qkv = np.asarray(w_qkv, dtype=np.float32)
    w_out = np.asarray(w_out, dtype=np.float32)
    mask = np.asarray(mask)

    # x pre-tiled: xt[p, g*4096 + kc*512 + c] = x[token g*512+c, kc*128+p]
    xt = np.ascontiguousarray(
        x.reshape(NG, 512, KC, 128).transpose(3, 0, 2, 1).reshape(128, NG * KC * 512)
    ).astype(bf)

    inv_freq = 1.0 / (ROPE_BASE ** (np.arange(0, HD, 2, dtype=np.float32) / HD))
    t = np.arange(T, dtype=np.float32)
    freqs = np.outer(t, inv_freq)                    # [T, 32]
    cos_r = np.cos(np.concatenate([freqs, freqs], 1)).T.astype(np.float32)  # [64, T]
    sin_half = np.sin(freqs).T.astype(np.float32)    # [32, T]
    sin_r = np.concatenate([-sin_half, sin_half], 0)  # [64, T] signed

    mb = np.zeros((128, B * NKT), dtype=np.float32)
    for b in range(B):
        for kt in range(NKT):
            mb[:, b * NKT + kt] = np.where(mask[b, kt * 128:(kt + 1) * 128], 0.0, -1e30)

    in_maps = []
    for c in range(NCORES):
        cs = slice(c * 128, (c + 1) * 128)
        blocks = []
        for kc in range(KC):
            kcs = slice(kc * 128, (kc + 1) * 128)
            blocks.append(np.concatenate(
                [w_qkv[kcs, 0:D][:, cs], w_qkv[kcs, D:2 * D][:, cs],
                 w_qkv[kcs, 2 * D:3 * D][:, cs]], axis=1))
        wq_c = np.ascontiguousarray(np.concatenate(blocks, axis=1)).astype(bf)
        if c == 0:
            cosc, sinc = cos_r, sin_r
        else:
            cosc = np.ones_like(cos_r)
            sinc = np.zeros_like(sin_r)
        in_maps.append({
            "xt": xt,
            "wqkv": wq_c,
            "wout": np.ascontiguousarray(w_out[cs, :]).astype(bf),
            "cosT": cosc.astype(bf),
            "sinT": sinc.astype(bf),
            "maskb": mb,
        })
    return in_maps


def kernel(x, w_qkv, w_out, mask):
    if "nc" not in _CACHE:
        _CACHE["nc"] = _build()
    nc = _CACHE["nc"]
    in_maps = _host_inputs(x, w_qkv, w_out, mask)

    from concourse.bass_utils import run_bass_kernel_spmd
    res = run_bass_kernel_spmd(nc, in_maps, core_ids=list(range(NCORES)))
    _CACHE["last_results"] = res

    total = np.zeros((NTOK, D), dtype=np.float32)
    for c in range(NCORES):
        part = np.asarray(res.results[c]["out"]).astype(np.float32)
        # out[p, g2*4096 + qt*1024 + nt*512 + f] -> token g2*512+qt*128+p
        total += part.reshape(128, NG, 4, 2, 512).transpose(1, 2, 0, 3, 4).reshape(NTOK, D)
    return total.reshape(B, T, D)
